# revision 4
# baseline (speedup 1.0000x reference)
"""Kuramoto layer on 8 Trainium2 NeuronCores (Bass/Tile, data-parallel over batch).

Math: the reference computes, per step,
    coupling[b,i] = sum_j K[i,j] * sin(theta[b,j] - theta[b,i])
which expands via sin(a-b) = sin(a)cos(b) - cos(a)sin(b) into
    coupling = cos(theta) * (S @ K^T) - sin(theta) * (C @ K^T)
so each step is two [128x256]@[256x256] matmuls + elementwise work instead of
the O(B*N^2) pairwise tensor. K_global*DT/N is folded into the matmul weights
on the host; DT*omega is applied per-partition via a fused scalar_tensor_tensor.

Device layout ("W"): per core a [128, 256] tile holds the transposed state:
    tile[p, h*128 + b] = m[osc = h*128 + p, batch = b],  m = theta + pi
Oscillators sit on partitions (2 halves along free dim) so the per-step
matmuls need no transposes at all. sin/cos come from the ACT `Sin` spline
(accurate only on ~[-pi-0.2, pi+0.2]), hence:
  - state m kept in [0, 2pi) and re-wrapped (mod 2pi) every 3rd step,
  - sin(theta) = Sin(m - pi),
  - cos(theta) = Sin(m2 - pi) with m2 = (m + pi/2) mod 2pi  (always in range).
Matmuls run in float32r (full-rate fp32 variant; requires N>=256, inputs
pre-rounded by the producing op). Final wrap to (-pi, pi] and the coherence
sqrt run on the host during unsharding.
"""
import numpy as np

BATCH = 1024
N_OSC = 256
DT = 0.1
STEPS = 10
N_CORES = 8
CB = BATCH // N_CORES  # 128 batch per core
H = N_OSC // 128       # 2 oscillator halves
WRAP_EVERY = 3

_PI = float(np.pi)
_2PI = float(2 * np.pi)

_cached_nc = None


def _build_program():
    global _cached_nc
    if _cached_nc is not None:
        return _cached_nc

    import concourse.tile as tile
    from concourse import bacc, mybir

    f32 = mybir.dt.float32
    f32r = mybir.dt.float32r
    Sin = mybir.ActivationFunctionType.Sin
    Abs = mybir.ActivationFunctionType.Abs
    add = mybir.AluOpType.add
    sub = mybir.AluOpType.subtract
    mult = mybir.AluOpType.mult

    nc = bacc.Bacc("TRN2", target_bir_lowering=False, debug=False)

    m_in = nc.dram_tensor("m0", [128, 256], f32, kind="ExternalInput").ap()
    k_in = nc.dram_tensor("k", [256, 256], f32, kind="ExternalInput").ap()
    om_in = nc.dram_tensor("om", [128, 2], f32, kind="ExternalInput").ap()
    m_out = nc.dram_tensor("m_out", [128, 256], f32, kind="ExternalOutput").ap()
    r2_out = nc.dram_tensor("r2_out", [1, 128], f32, kind="ExternalOutput").ap()

    def s_blocks(ap512):  # [S_0 | C_0 | S_1 | C_1] -> S blocks as [128, 2, 128]
        return ap512.rearrange("p (h two c) -> p h two c", two=2, c=128)[:, :, 0, :]

    def c_blocks(ap512):
        return ap512.rearrange("p (h two c) -> p h two c", two=2, c=128)[:, :, 1, :]

    def halves(ap256):  # [128, 256] -> [128, 2, 128]
        return ap256.rearrange("p (h c) -> p h c", c=128)

    with tile.TileContext(nc) as tc:
        with tc.tile_pool(name="const", bufs=1) as cpool, \
             tc.tile_pool(name="work", bufs=2) as wpool, \
             tc.tile_pool(name="psum", bufs=2, space="PSUM") as ppool, \
             tc.tile_pool(name="psum_red", bufs=1, space="PSUM") as rpool:

            # ---- constants / inputs ----
            kstage = cpool.tile([128, 512], f32, tag="kstage")
            nc.sync.dma_start(kstage[:, 0:256], k_in[0:128, :])
            nc.sync.dma_start(kstage[:, 256:512], k_in[128:256, :])
            kt = cpool.tile([128, 512], f32r, tag="kt")
            nc.vector.tensor_copy(kt[:], kstage[:])

            om_t = cpool.tile([128, 2], f32, tag="om")
            nc.sync.dma_start(om_t[:], om_in[:])

            b_zero = cpool.tile([128, 1], f32, tag="bzero")
            nc.vector.memset(b_zero[:], 0.0)
            b_hpi = cpool.tile([128, 1], f32, tag="bhpi")
            nc.vector.memset(b_hpi[:], _PI / 2)
            ones_f = cpool.tile([128, 1], f32, tag="ones_f")
            nc.vector.memset(ones_f[:], 1.0)
            ones_r = cpool.tile([128, 1], f32r, tag="ones")
            nc.vector.tensor_copy(ones_r[:], ones_f[:])

            m = cpool.tile([128, 256], f32, tag="m_state")
            nc.sync.dma_start(m[:], m_in[:])

            for s in range(STEPS):
                # CS = [sin | cos] interleaved by half: [S_0 | C_0 | S_1 | C_1]
                # cos(t) = sin(pi/2 - |t|); |t| <= pi + drift keeps args in-domain
                ab = wpool.tile([128, 256], f32, tag="ab")
                nc.scalar.activation(ab[:], m[:], Abs, bias=b_zero[:])
                cs = wpool.tile([128, 512], f32r, tag="cs")
                nc.scalar.activation(s_blocks(cs[:]), halves(m[:]), Sin,
                                     bias=b_zero[:])
                nc.scalar.activation(c_blocks(cs[:]), halves(ab[:]), Sin,
                                     bias=b_hpi[:], scale=-1.0)

                # psum = [MS_0 | MC_0 | MS_1 | MC_1]; accumulate over j-halves
                ps = ppool.tile([128, 512], f32, tag="ps")
                for g in range(H):
                    for h in range(H):
                        nc.tensor.matmul(
                            ps[:, g * 256:(g + 1) * 256],
                            kt[:, h * 256 + g * 128: h * 256 + g * 128 + 128],
                            cs[:, h * 256:(h + 1) * 256],
                            start=(h == 0), stop=(h == H - 1),
                        )

                # P1 = cos .* MS ; P2 = sin .* MC   (strided block views)
                p1 = wpool.tile([128, 256], f32, tag="p1")
                nc.vector.tensor_tensor(halves(p1[:]), c_blocks(cs[:]),
                                        s_blocks(ps[:]), mult)
                p2 = wpool.tile([128, 256], f32, tag="p2")
                nc.vector.tensor_tensor(halves(p2[:]), s_blocks(cs[:]),
                                        c_blocks(ps[:]), mult)

                # t = (P1 + DT*omega) - P2 per half, then m += t
                t = wpool.tile([128, 256], f32, tag="t")
                for h in range(H):
                    nc.vector.scalar_tensor_tensor(
                        t[:, h * 128:(h + 1) * 128],
                        p1[:, h * 128:(h + 1) * 128],
                        om_t[:, h:h + 1],
                        p2[:, h * 128:(h + 1) * 128],
                        add, sub,
                    )
                m_new = wpool.tile([128, 256], f32, tag="m_state_next")
                nc.vector.tensor_tensor(m_new[:], m[:], t[:], add)
                m = m_new

            # ---- coherence: r2 = (sum_i sin)^2 + (sum_i cos)^2 per batch ----
            abf = wpool.tile([128, 256], f32, tag="ab")
            nc.scalar.activation(abf[:], m[:], Abs, bias=b_zero[:])
            csf = wpool.tile([128, 512], f32r, tag="cs")
            nc.scalar.activation(s_blocks(csf[:]), halves(m[:]), Sin,
                                 bias=b_zero[:])
            nc.scalar.activation(c_blocks(csf[:]), halves(abf[:]), Sin,
                                 bias=b_hpi[:], scale=-1.0)
            red = rpool.tile([1, 512], f32, tag="red")
            nc.tensor.matmul(red[:], ones_r[:], csf[:], start=True, stop=True)
            red_sb = wpool.tile([1, 512], f32, tag="red_sb")
            nc.vector.tensor_copy(red_sb[:], red[:])
            sums = wpool.tile([1, 256], f32, tag="sums")
            nc.vector.tensor_tensor(sums[:], red_sb[0:1, 0:256],
                                    red_sb[0:1, 256:512], add)
            sq = wpool.tile([1, 256], f32, tag="sq")
            nc.vector.tensor_tensor(sq[:], sums[:], sums[:], mult)
            r2 = wpool.tile([1, 128], f32, tag="r2")
            nc.vector.tensor_tensor(r2[:], sq[0:1, 0:128], sq[0:1, 128:256], add)

            nc.sync.dma_start(m_out[:], m[:])
            nc.sync.dma_start(r2_out[:], r2[:])

    nc.compile()
    _cached_nc = nc
    return nc


def kernel(theta_init, K, omega, K_global):
    from concourse.bass_utils import run_bass_kernel_spmd

    nc = _build_program()

    th = np.asarray(theta_init, dtype=np.float32)
    Kf = np.asarray(K, dtype=np.float64)
    om = np.asarray(omega, dtype=np.float64)
    kg = float(np.asarray(K_global))

    # state = theta wrapped to [-pi, pi); folded coupling scale into lhsT
    t64 = th.astype(np.float64)
    m0 = (t64 - 2 * np.pi * np.round(t64 / (2 * np.pi))).astype(np.float32)
    k_lhsT = np.ascontiguousarray((DT * kg / N_OSC) * Kf.T).astype(np.float32)
    om2 = np.ascontiguousarray((DT * om).astype(np.float32).reshape(2, 128).T)

    in_maps = []
    for c in range(N_CORES):
        sl = m0[c * CB:(c + 1) * CB, :]          # [128 b, 256 osc]
        t = sl.T                                  # [256 osc, 128 b]
        w = np.ascontiguousarray(
            np.concatenate([t[0:128, :], t[128:256, :]], axis=1))  # [128, 256]
        in_maps.append({"m0": w, "k": k_lhsT, "om": om2})

    res = run_bass_kernel_spmd(nc, in_maps, core_ids=list(range(N_CORES))).results

    theta = np.empty((BATCH, N_OSC), dtype=np.float32)
    coherence = np.empty((BATCH,), dtype=np.float32)
    for c in range(N_CORES):
        mw = res[c]["m_out"]                      # [128, 256] layout W
        tr = np.concatenate([mw[:, 0:128], mw[:, 128:256]], axis=0)  # [256, 128]
        m_full = tr.T.astype(np.float64)          # [128 b, 256 osc]
        w64 = m_full - 2 * np.pi * np.round(m_full / (2 * np.pi))
        theta[c * CB:(c + 1) * CB, :] = w64.astype(np.float32)
        r2 = res[c]["r2_out"].reshape(-1)[0:CB].astype(np.float64)
        coherence[c * CB:(c + 1) * CB] = (np.sqrt(np.maximum(r2, 0.0)) / N_OSC
                                          ).astype(np.float32)
    return theta, coherence


# revision 5
# speedup vs baseline: 7380.5123x; 7380.5123x over previous
"""Kuramoto layer on 8 Trainium2 NeuronCores (Bass/Tile, data-parallel over batch).

Math: the reference computes, per step,
    coupling[b,i] = sum_j K[i,j] * sin(theta[b,j] - theta[b,i])
which expands via sin(a-b) = sin(a)cos(b) - cos(a)sin(b) into
    coupling = cos(theta) * (S @ K^T) - sin(theta) * (C @ K^T)
so each step is two [128x256]@[256x256] matmuls + elementwise work instead of
the O(B*N^2) pairwise tensor. K_global*DT/N is folded into the matmul weights
on the host; DT*omega is applied per-partition via a fused scalar_tensor_tensor.

Device layout ("W"): per core a [128, 256] tile holds the transposed state:
    tile[p, h*128 + b] = m[osc = h*128 + p, batch = b],  m = theta + pi
Oscillators sit on partitions (2 halves along free dim) so the per-step
matmuls need no transposes at all. sin/cos come from the ACT `Sin` spline
(accurate only on ~[-pi-0.2, pi+0.2]), hence:
  - state m kept in [0, 2pi) and re-wrapped (mod 2pi) every 3rd step,
  - sin(theta) = Sin(m - pi),
  - cos(theta) = Sin(m2 - pi) with m2 = (m + pi/2) mod 2pi  (always in range).
Matmuls run in float32r (full-rate fp32 variant; requires N>=256, inputs
pre-rounded by the producing op). Final wrap to (-pi, pi] and the coherence
sqrt run on the host during unsharding.
"""
import numpy as np

BATCH = 1024
N_OSC = 256
DT = 0.1
STEPS = 10
N_CORES = 8
CB = BATCH // N_CORES  # 128 batch per core
H = N_OSC // 128       # 2 oscillator halves
WRAP_EVERY = 3

_PI = float(np.pi)
_2PI = float(2 * np.pi)

_cached_nc = None


def _build_program():
    global _cached_nc
    if _cached_nc is not None:
        return _cached_nc

    import concourse.tile as tile
    from concourse import bacc, mybir

    f32 = mybir.dt.float32
    f32r = mybir.dt.float32r
    Sin = mybir.ActivationFunctionType.Sin
    Abs = mybir.ActivationFunctionType.Abs
    add = mybir.AluOpType.add
    sub = mybir.AluOpType.subtract
    mult = mybir.AluOpType.mult

    nc = bacc.Bacc("TRN2", target_bir_lowering=False, debug=False)

    m_in = nc.dram_tensor("m0", [128, 256], f32, kind="ExternalInput").ap()
    k_in = nc.dram_tensor("k", [256, 256], f32, kind="ExternalInput").ap()
    om_in = nc.dram_tensor("om", [128, 2], f32, kind="ExternalInput").ap()
    m_out = nc.dram_tensor("m_out", [128, 256], f32, kind="ExternalOutput").ap()
    r2_out = nc.dram_tensor("r2_out", [1, 128], f32, kind="ExternalOutput").ap()

    def s_blocks(ap512):  # [S_0 | C_0 | S_1 | C_1] -> S blocks as [128, 2, 128]
        return ap512.rearrange("p (h two c) -> p h two c", two=2, c=128)[:, :, 0, :]

    def c_blocks(ap512):
        return ap512.rearrange("p (h two c) -> p h two c", two=2, c=128)[:, :, 1, :]

    def halves(ap256):  # [128, 256] -> [128, 2, 128]
        return ap256.rearrange("p (h c) -> p h c", c=128)

    with tile.TileContext(nc) as tc:
        with tc.tile_pool(name="const", bufs=1) as cpool, \
             tc.tile_pool(name="work", bufs=2) as wpool, \
             tc.tile_pool(name="psum", bufs=2, space="PSUM") as ppool, \
             tc.tile_pool(name="psum_red", bufs=1, space="PSUM") as rpool:

            # ---- constants / inputs ----
            kstage = cpool.tile([128, 512], f32, tag="kstage")
            nc.sync.dma_start(kstage[:, 0:256], k_in[0:128, :])
            nc.sync.dma_start(kstage[:, 256:512], k_in[128:256, :])
            kt = cpool.tile([128, 512], f32r, tag="kt")
            nc.vector.tensor_copy(kt[:], kstage[:])

            om_t = cpool.tile([128, 2], f32, tag="om")
            nc.sync.dma_start(om_t[:], om_in[:])

            b_zero = cpool.tile([128, 1], f32, tag="bzero")
            nc.vector.memset(b_zero[:], 0.0)
            b_hpi = cpool.tile([128, 1], f32, tag="bhpi")
            nc.vector.memset(b_hpi[:], _PI / 2)
            ones_f = cpool.tile([128, 1], f32, tag="ones_f")
            nc.vector.memset(ones_f[:], 1.0)
            ones_r = cpool.tile([128, 1], f32r, tag="ones")
            nc.vector.tensor_copy(ones_r[:], ones_f[:])

            m = cpool.tile([128, 256], f32, tag="m_state")
            nc.sync.dma_start(m[:], m_in[:])

            for s in range(STEPS):
                # CS = [sin | cos] interleaved by half: [S_0 | C_0 | S_1 | C_1]
                # cos(t) = sin(pi/2 - |t|); |t| <= pi + drift keeps args in-domain
                ab = wpool.tile([128, 256], f32, tag="ab")
                nc.scalar.activation(ab[:], m[:], Abs, bias=b_zero[:])
                cs = wpool.tile([128, 512], f32r, tag="cs")
                nc.scalar.activation(s_blocks(cs[:]), halves(m[:]), Sin,
                                     bias=b_zero[:])
                nc.scalar.activation(c_blocks(cs[:]), halves(ab[:]), Sin,
                                     bias=b_hpi[:], scale=-1.0)

                # psum = [MS_0 | MC_0 | MS_1 | MC_1]; accumulate over j-halves
                ps = ppool.tile([128, 512], f32, tag="ps")
                for g in range(H):
                    for h in range(H):
                        nc.tensor.matmul(
                            ps[:, g * 256:(g + 1) * 256],
                            kt[:, h * 256 + g * 128: h * 256 + g * 128 + 128],
                            cs[:, h * 256:(h + 1) * 256],
                            start=(h == 0), stop=(h == H - 1),
                        )

                # prod = [C0*MS0 | S0*MC0 | C1*MS1 | S1*MC1] in ONE wide TT:
                # cs viewed with each half's (S,C) pair swapped -> [C0,S0,C1,S1]
                cs_sw = cs[:].rearrange("p (h two c) -> p h two c",
                                        two=2, c=128)[:, :, ::-1, :]
                prod = wpool.tile([128, 512], f32, tag="prod")
                nc.vector.tensor_tensor(
                    prod[:].rearrange("p (h two c) -> p h two c", two=2, c=128),
                    cs_sw, ps[:].rearrange("p (h two c) -> p h two c",
                                           two=2, c=128), mult)

                # t = (P1 + DT*omega) - P2 per half, then m += t
                t = wpool.tile([128, 256], f32, tag="t")
                for h in range(H):
                    nc.vector.scalar_tensor_tensor(
                        t[:, h * 128:(h + 1) * 128],
                        prod[:, h * 256:h * 256 + 128],
                        om_t[:, h:h + 1],
                        prod[:, h * 256 + 128:(h + 1) * 256],
                        add, sub,
                    )
                m_new = wpool.tile([128, 256], f32, tag="m_state_next")
                nc.vector.tensor_tensor(m_new[:], m[:], t[:], add)
                m = m_new

            # ---- coherence: r2 = (sum_i sin)^2 + (sum_i cos)^2 per batch ----
            abf = wpool.tile([128, 256], f32, tag="ab")
            nc.scalar.activation(abf[:], m[:], Abs, bias=b_zero[:])
            csf = wpool.tile([128, 512], f32r, tag="cs")
            nc.scalar.activation(s_blocks(csf[:]), halves(m[:]), Sin,
                                 bias=b_zero[:])
            nc.scalar.activation(c_blocks(csf[:]), halves(abf[:]), Sin,
                                 bias=b_hpi[:], scale=-1.0)
            red = rpool.tile([1, 512], f32, tag="red")
            nc.tensor.matmul(red[:], ones_r[:], csf[:], start=True, stop=True)
            red_sb = wpool.tile([1, 512], f32, tag="red_sb")
            nc.vector.tensor_copy(red_sb[:], red[:])
            sums = wpool.tile([1, 256], f32, tag="sums")
            nc.vector.tensor_tensor(sums[:], red_sb[0:1, 0:256],
                                    red_sb[0:1, 256:512], add)
            sq = wpool.tile([1, 256], f32, tag="sq")
            nc.vector.tensor_tensor(sq[:], sums[:], sums[:], mult)
            r2 = wpool.tile([1, 128], f32, tag="r2")
            nc.vector.tensor_tensor(r2[:], sq[0:1, 0:128], sq[0:1, 128:256], add)

            nc.sync.dma_start(m_out[:], m[:])
            nc.sync.dma_start(r2_out[:], r2[:])

    nc.compile()
    _cached_nc = nc
    return nc


def kernel(theta_init, K, omega, K_global):
    from concourse.bass_utils import run_bass_kernel_spmd

    nc = _build_program()

    th = np.asarray(theta_init, dtype=np.float32)
    Kf = np.asarray(K, dtype=np.float64)
    om = np.asarray(omega, dtype=np.float64)
    kg = float(np.asarray(K_global))

    # state = theta wrapped to [-pi, pi); folded coupling scale into lhsT
    t64 = th.astype(np.float64)
    m0 = (t64 - 2 * np.pi * np.round(t64 / (2 * np.pi))).astype(np.float32)
    k_lhsT = np.ascontiguousarray((DT * kg / N_OSC) * Kf.T).astype(np.float32)
    om2 = np.ascontiguousarray((DT * om).astype(np.float32).reshape(2, 128).T)

    in_maps = []
    for c in range(N_CORES):
        sl = m0[c * CB:(c + 1) * CB, :]          # [128 b, 256 osc]
        t = sl.T                                  # [256 osc, 128 b]
        w = np.ascontiguousarray(
            np.concatenate([t[0:128, :], t[128:256, :]], axis=1))  # [128, 256]
        in_maps.append({"m0": w, "k": k_lhsT, "om": om2})

    res = run_bass_kernel_spmd(nc, in_maps, core_ids=list(range(N_CORES))).results

    theta = np.empty((BATCH, N_OSC), dtype=np.float32)
    coherence = np.empty((BATCH,), dtype=np.float32)
    for c in range(N_CORES):
        mw = res[c]["m_out"]                      # [128, 256] layout W
        tr = np.concatenate([mw[:, 0:128], mw[:, 128:256]], axis=0)  # [256, 128]
        m_full = tr.T.astype(np.float64)          # [128 b, 256 osc]
        w64 = m_full - 2 * np.pi * np.round(m_full / (2 * np.pi))
        theta[c * CB:(c + 1) * CB, :] = w64.astype(np.float32)
        r2 = res[c]["r2_out"].reshape(-1)[0:CB].astype(np.float64)
        coherence[c * CB:(c + 1) * CB] = (np.sqrt(np.maximum(r2, 0.0)) / N_OSC
                                          ).astype(np.float32)
    return theta, coherence
